# revision 25
# baseline (speedup 1.0000x reference)
"""Edge-parallel ExtractorMLP (gather + 3-layer MLP) for 8 TRN2 NeuronCores.

Strategy (pure edge parallelism, no sorting, no cross-core communication):
  - Core c takes the contiguous edge slice [c*100000, (c+1)*100000), padded
    to 196 tiles of 512 edges. Edge order is preserved end to end (modulo a
    few host-side swaps, undone on unshard).
  - BOTH endpoints are fetched with tile-aligned SWDGE dma_gather calls (512
    indices each) from the fp16 [50000, 128] embedding table in HBM. Indices
    are int16 offsets from a base at row 25000 (signed DMA address math
    covers the full +/-25000 range, verified on HW); per call the last 16
    slots are host-swapped to edges whose both endpoints are >= 25000,
    because the ucode clamps a trailing run of negative indices. Gathers
    land feature-major [128, 512] fp16 in SBUF -- directly usable as matmul
    rhs, so the whole one-hot/PE-gather machinery of the previous version is
    gone.
  - Desc-gen runs at ~9ns/index per SWDGE queue; four queues (round-robin
    per call) give ~3.3x aggregate throughput, hiding the 200K-index gather
    under the PE time. Multi-queue first-use has a cold-start corruption
    transient, neutralized by 8 sacrificial 128-idx gathers per queue before
    any real call (see the warmup block; 18/18 clean soak runs). Gathers are
    emitted per group of 14 tiles, double-buffered, overlapping the MLP of
    the previous group.
  - The MLP runs per 512-edge tile on the tensor engine in fp16 with fp32
    PSUM accumulation: layer 1 as 4 M-chunks x 2 K-chunks, layer 2 as 4
    K-chunks, layer 3 as a single [128,1] stationary matmul per tile writing
    its own partition row of a per-group [14, 512] PSUM tile. Bias+ReLU
    epilogues are split between the scalar (ACT: L1 m0-m2) and vector (DVE:
    L1 m3, L2) engines; layer 3 bias and the output DMA are per-group.
"""

from contextlib import ExitStack

import numpy as np

import concourse.bacc as bacc
import concourse.tile as tile
from concourse import mybir
from concourse.bass_utils import run_bass_kernel_spmd

P = 128
N = 512              # edges per tile (one fp32 PSUM bank)
CALL = 512           # indices per dma_gather call (tile-aligned)
IDXW = CALL // 16    # wrapped-index columns per call
TPG = 14             # tiles per group (buffering unit)
NG = 14              # groups
N_TILES = NG * TPG   # 196
NCALLS = N_TILES * N // CALL  # 196 per endpoint
SLOTS = N_TILES * N  # 100352
N_CORES = 8
N_NODES = 50000
BASE = 25000         # gather base row (centered; offsets fit int16)
N_EDGES = 800000
E_CORE = N_EDGES // N_CORES

F16 = mybir.dt.float16
F32 = mybir.dt.float32
I16 = mybir.dt.int16


N_QUEUES = 4  # SWDGE queues; desc-gen parallelizes across Q7 cores


def _build_kernel():
    nc = bacc.Bacc("TRN2", target_bir_lowering=False, debug=False,
                   num_swdge_queues=N_QUEUES,
                   # 4 queues share the SWDGE descriptor carveout; the default
                   # 16KB overflows (clobbered descriptors -> lane-structured
                   # garbage gathers) when desc-gen runs ahead of DMA drain.
                   dynamic_dma_scratch_size=65536)

    tbl = nc.dram_tensor("tbl", [N_NODES, P], F16, kind="ExternalInput")
    colw = nc.dram_tensor("colw", [P, NCALLS * IDXW], I16, kind="ExternalInput")
    roww = nc.dram_tensor("roww", [P, NCALLS * IDXW], I16, kind="ExternalInput")
    w1 = nc.dram_tensor("w1", [P, 1024], F16, kind="ExternalInput")
    w2 = nc.dram_tensor("w2", [P, 512], F16, kind="ExternalInput")
    w3 = nc.dram_tensor("w3", [P, 1], F16, kind="ExternalInput")
    b1 = nc.dram_tensor("b1", [P, 4], F32, kind="ExternalInput")
    b2 = nc.dram_tensor("b2", [P, 1], F32, kind="ExternalInput")
    b3 = nc.dram_tensor("b3", [P, 1], F32, kind="ExternalInput")
    out = nc.dram_tensor("out", [1, N_TILES * N], F32, kind="ExternalOutput")

    Relu = mybir.ActivationFunctionType.Relu
    Identity = mybir.ActivationFunctionType.Identity
    Op = mybir.AluOpType

    with tile.TileContext(nc) as tc, ExitStack() as ctx:
        idxp = ctx.enter_context(tc.tile_pool(name="idxp", bufs=1))
        wp = ctx.enter_context(tc.tile_pool(name="wp", bufs=1))
        gcp = ctx.enter_context(tc.tile_pool(name="gcp", bufs=2 * TPG))
        grp = ctx.enter_context(tc.tile_pool(name="grp", bufs=2 * TPG))
        x1p = ctx.enter_context(tc.tile_pool(name="x1p", bufs=12))
        x2p = ctx.enter_context(tc.tile_pool(name="x2p", bufs=4))
        op = ctx.enter_context(tc.tile_pool(name="op", bufs=1))
        pl1 = ctx.enter_context(tc.tile_pool(name="pl1", bufs=4, space="PSUM"))
        pl2 = ctx.enter_context(tc.tile_pool(name="pl2", bufs=2, space="PSUM"))
        pl3 = ctx.enter_context(tc.tile_pool(name="pl3", bufs=2, space="PSUM"))

        # ---- one-time loads -------------------------------------------
        colw_sb = idxp.tile([P, NCALLS * IDXW], I16)
        roww_sb = idxp.tile([P, NCALLS * IDXW], I16)
        nc.scalar.dma_start(colw_sb[:], colw[:])
        nc.scalar.dma_start(roww_sb[:], roww[:])

        w1_sb = wp.tile([P, 1024], F16)
        w2_sb = wp.tile([P, 512], F16)
        w3_sb = wp.tile([P, 1], F16)
        b1_sb = wp.tile([P, 4], F32)
        b2_sb = wp.tile([P, 1], F32)
        b3_sb = wp.tile([P, 1], F32)
        nc.scalar.dma_start(w1_sb[:], w1[:])
        nc.scalar.dma_start(w2_sb[:], w2[:])
        nc.scalar.dma_start(w3_sb[:], w3[:])
        nc.scalar.dma_start(b1_sb[:], b1[:])
        nc.scalar.dma_start(b2_sb[:], b2[:])
        nc.scalar.dma_start(b3_sb[:], b3[:])

        tblc = tbl[BASE:N_NODES, :]  # centered base; signed offsets

        # Pool-queue guard: SWDGE desc-gen reads index VALUES from SBUF, and
        # the HWDGE idx-load -> Q7 desc-gen dependency has been observed to
        # race on HW. A Pool-engine read of one column of each idx buffer
        # stalls the Pool queue on the idx DMA completion sems, so every
        # later dma_gather is safely behind the loads.
        chk = wp.tile([P, 2], F16)
        nc.gpsimd.tensor_scalar(
            out=chk[:, 0:1], in0=colw_sb[:, 0:1].bitcast(F16),
            scalar1=0.0, scalar2=None, op0=Op.add)
        nc.gpsimd.tensor_scalar(
            out=chk[:, 1:2], in0=roww_sb[:, 0:1].bitcast(F16),
            scalar1=0.0, scalar2=None, op0=Op.add)

        # Sacrificial queue warmup: every observed multi-queue corruption
        # hit only the first ~16 gather calls after start (cold per-queue
        # ring/ucode state). Burn 8 dummy 128-idx gathers per queue into a
        # scratch tile, then serialize, so the transient cannot touch real
        # data.
        # Zeroed warmup indices (offset 0 = table row BASE, a valid read):
        # the warmup then has no dependency on the idx-load DMAs and runs
        # concurrently with them. Separate scratch tiles per call keep the
        # per-queue warmup chains free of WAW serialization.
        warm_idx = wp.tile([P, 8], I16)
        nc.gpsimd.memset(warm_idx[:], 0)
        warms = [wp.tile([P, 1, 128], F16, name=f"warm{w}")
                 for w in range(8 * N_QUEUES)]
        for w in range(8):
            for q in range(N_QUEUES):
                nc.gpsimd.dma_gather(
                    warms[w * N_QUEUES + q][:], tblc, warm_idx[:], 128, 128,
                    P, transpose=True, queue_num=q,
                )
        for w in range(8 * N_QUEUES):
            nc.gpsimd.tensor_scalar(
                out=chk[:, 0:1], in0=warms[w][:, 0, 0:1],
                scalar1=0.0, scalar2=None, op0=Op.add)

        qn = [0]

        def emit_group_gathers(g):
            tiles = []
            for j in range(TPG):
                k = g * TPG + j
                gc = gcp.tile([P, 1, N], F16, tag="gc", name=f"gc{k}")
                gr = grp.tile([P, 1, N], F16, tag="gr", name=f"gr{k}")
                for w_sb, gt in ((colw_sb, gc), (roww_sb, gr)):
                    nc.gpsimd.dma_gather(
                        gt[:], tblc,
                        w_sb[:, k * IDXW:(k + 1) * IDXW], CALL, CALL, P,
                        transpose=True, queue_num=qn[0] % N_QUEUES,
                    )
                    qn[0] += 1
                tiles.append((gc, gr))
            return tiles

        gabs = {0: emit_group_gathers(0)}

        for g in range(NG):
            if g + 1 < NG:
                gabs[g + 1] = emit_group_gathers(g + 1)
            tiles = gabs.pop(g)

            o = op.tile([1, TPG * N], F32, tag="o", name=f"o{g}")
            # Consume the two newest tiles first: their 4 gathers are the
            # last call on each SWDGE queue, and per-queue FIFO completion
            # then guarantees every earlier gather of the group has landed
            # (the Tile lane-sem waits assume in-order completion, which 4
            # concurrent queues otherwise break at startup).
            for tt in [TPG - 1, TPG - 2] + list(range(TPG - 2)):
                gc, gr = tiles[tt]
                rc = gc[:, 0, :]
                rr = gr[:, 0, :]

                # layer 1: [E,256] @ [256,512]; K-chunk 0 = col, 1 = row
                x1s = []
                for m in range(4):
                    p1 = pl1.tile([P, N], F32, tag="pl1")
                    nc.tensor.matmul(
                        p1[:], lhsT=w1_sb[:, m * 128:(m + 1) * 128],
                        rhs=rc, start=True, stop=False,
                    )
                    nc.tensor.matmul(
                        p1[:], lhsT=w1_sb[:, 512 + m * 128:512 + (m + 1) * 128],
                        rhs=rr, start=False, stop=True,
                    )
                    x1 = x1p.tile([P, N], F16, tag="x1")
                    if m < 3:
                        nc.scalar.activation(
                            x1[:], p1[:], Relu, bias=b1_sb[:, m:m + 1]
                        )
                    else:
                        nc.vector.tensor_scalar(
                            out=x1[:], in0=p1[:],
                            scalar1=b1_sb[:, m:m + 1], scalar2=0.0,
                            op0=Op.add, op1=Op.max,
                        )
                    x1s.append(x1)

                # layer 2: [E,512] @ [512,128]
                p2 = pl2.tile([P, N], F32, tag="pl2")
                for k in range(4):
                    nc.tensor.matmul(
                        p2[:], lhsT=w2_sb[:, k * 128:(k + 1) * 128],
                        rhs=x1s[k][:], start=(k == 0), stop=(k == 3),
                    )
                x2 = x2p.tile([P, N], F16, tag="x2")
                nc.vector.tensor_scalar(
                    out=x2[:], in0=p2[:],
                    scalar1=b2_sb[:, 0:1], scalar2=0.0,
                    op0=Op.add, op1=Op.max,
                )

                # layer 3: [E,128] @ [128,1]; bias lands row tt of the
                # group staging tile so the output DMA is per-group
                p3 = pl3.tile([1, N], F32, tag="pl3")
                nc.tensor.matmul(p3[:1, :], lhsT=w3_sb[:], rhs=x2[:],
                                 start=True, stop=True)
                nc.scalar.activation(o[0:1, tt * N:(tt + 1) * N], p3[:1, :],
                                     Identity, bias=b3_sb[0:1, 0:1])

            nc.sync.dma_start(
                out[0:1, g * TPG * N:(g + 1) * TPG * N], o[:])

    nc.compile()
    return nc


def _wrap_indices(offs: np.ndarray) -> np.ndarray:
    """[NCALLS*CALL] int16 offsets -> [128, NCALLS*IDXW] wrapped layout.

    dma_gather unwraps each 16-partition group as rearrange("p s -> (s p)")
    per call, so index j of call k sits at [16g + j%16, k*IDXW + j//16],
    replicated over the 8 groups g.
    """
    w = offs.reshape(NCALLS, IDXW, 16).transpose(0, 2, 1)  # [NCALLS, 16, IDXW]
    w = np.tile(w, (1, 8, 1))                              # [NCALLS, 128, IDXW]
    return np.ascontiguousarray(
        w.transpose(1, 0, 2).reshape(P, NCALLS * IDXW))


def _plan_core(col: np.ndarray, row: np.ndarray, base_slot: int):
    """Pad a core's contiguous edge slice to SLOTS and fix call tails.

    The ucode clamps a trailing run of negative int16 indices in each
    dma_gather call, so the last 16 slots of every 896-slot call must hold
    edges with BOTH endpoints >= BASE (offset >= 0). Swap such edges into
    the tail; `so` records each slot's original edge id (-1 = pad).

    Returns (colw [128, NCALLS*IDXW] i16, roww likewise, so [SLOTS] i64).
    """
    n = col.shape[0]
    oc = np.full(SLOTS, 0, np.int64)
    orr = np.full(SLOTS, 0, np.int64)
    so = np.full(SLOTS, -1, np.int64)
    oc[:n] = col - BASE
    orr[:n] = row - BASE
    so[:n] = base_slot + np.arange(n)

    both = (oc >= 0) & (orr >= 0)
    for k in range(NCALLS):
        s = k * CALL
        tail = np.arange(s + CALL - 16, s + CALL)
        tail = tail[~both[tail]]
        if tail.size == 0:
            continue
        cand = s + np.flatnonzero(both[s:s + CALL - 16])
        assert cand.size >= tail.size, "no non-negative tail candidates"
        cand = cand[:tail.size]
        for arr in (oc, orr, so, both):
            arr[tail], arr[cand] = arr[cand], arr[tail]
    assert oc.min() >= -BASE and oc.max() < N_NODES - BASE
    assert orr.min() >= -BASE and orr.max() < N_NODES - BASE
    return (_wrap_indices(oc.astype(np.int16)),
            _wrap_indices(orr.astype(np.int16)), so)


def _prep_shared(emb, W1, b1, W2, b2, W3, b3):
    return {
        "tbl": np.ascontiguousarray(emb.astype(np.float16)),
        "w1": np.ascontiguousarray(
            np.concatenate([W1[:128, :], W1[128:, :]], axis=1)
        ).astype(np.float16),
        "w2": np.ascontiguousarray(
            np.concatenate([W2[k * 128:(k + 1) * 128, :] for k in range(4)],
                           axis=1)
        ).astype(np.float16),
        "w3": W3.astype(np.float16),
        "b1": np.ascontiguousarray(b1.reshape(4, 128).T).astype(np.float32),
        "b2": b2[:, None].astype(np.float32),
        "b3": np.broadcast_to(b3[None, :], (P, 1)).astype(np.float32).copy(),
    }


_NC_CACHE = {}


def _get_nc():
    if "nc" not in _NC_CACHE:
        _NC_CACHE["nc"] = _build_kernel()
    return _NC_CACHE["nc"]


def run(inputs: dict, trace: bool = False):
    """Run the kernel on 8 cores; returns (out [800000,1] f32, results)."""
    emb = np.asarray(inputs["emb"], dtype=np.float32)
    edge_index = np.asarray(inputs["edge_index"])
    shared = _prep_shared(
        emb,
        *[np.asarray(inputs[k], dtype=np.float32)
          for k in ("W1", "b1", "W2", "b2", "W3", "b3")]
    )
    col = np.asarray(edge_index[0], dtype=np.int64)
    row = np.asarray(edge_index[1], dtype=np.int64)

    in_maps = []
    sos = []
    for c in range(N_CORES):
        cw, rw, so = _plan_core(
            col[c * E_CORE:(c + 1) * E_CORE],
            row[c * E_CORE:(c + 1) * E_CORE],
            c * E_CORE,
        )
        in_maps.append(dict(shared, colw=cw, roww=rw))
        sos.append(so)

    nc = _get_nc()
    res = run_bass_kernel_spmd(nc, in_maps, list(range(N_CORES)), trace=trace)
    out = np.empty((N_EDGES,), np.float32)
    for c in range(N_CORES):
        flat = res.results[c]["out"].reshape(-1)
        so = sos[c]
        valid = so >= 0
        out[so[valid]] = flat[valid]
    return out[:, None], res


def kernel(**inputs) -> np.ndarray:
    out, _ = run(inputs, trace=False)
    return out


# revision 26
# speedup vs baseline: 1.0966x; 1.0966x over previous
"""Edge-parallel ExtractorMLP (gather + 3-layer MLP) for 8 TRN2 NeuronCores.

Strategy (pure edge parallelism, no sorting, no cross-core communication):
  - Core c takes the contiguous edge slice [c*100000, (c+1)*100000), padded
    to 196 tiles of 512 edges. Edge order is preserved end to end (modulo a
    few host-side swaps, undone on unshard).
  - BOTH endpoints are fetched with tile-aligned SWDGE dma_gather calls (512
    indices each) from the fp16 [50000, 128] embedding table in HBM. Indices
    are int16 offsets from a base at row 25000 (signed DMA address math
    covers the full +/-25000 range, verified on HW); per call the last 16
    slots are host-swapped to edges whose both endpoints are >= 25000,
    because the ucode clamps a trailing run of negative indices. Gathers
    land feature-major [128, 512] fp16 in SBUF -- directly usable as matmul
    rhs, so the whole one-hot/PE-gather machinery of the previous version is
    gone.
  - Desc-gen runs at ~9ns/index per SWDGE queue; four queues (round-robin
    per call) give ~3.3x aggregate throughput, hiding the 200K-index gather
    under the PE time. Multi-queue first-use has a cold-start corruption
    transient, neutralized by 8 sacrificial 128-idx gathers per queue before
    any real call (see the warmup block; 18/18 clean soak runs). Gathers are
    emitted per group of 14 tiles, double-buffered, overlapping the MLP of
    the previous group.
  - The MLP runs per 512-edge tile on the tensor engine in fp16 with fp32
    PSUM accumulation: layer 1 as 4 M-chunks x 2 K-chunks, layer 2 as 4
    K-chunks, layer 3 as a single [128,1] stationary matmul per tile writing
    its own partition row of a per-group [14, 512] PSUM tile. Bias+ReLU
    epilogues are split between the scalar (ACT: L1 m0-m2) and vector (DVE:
    L1 m3, L2) engines; layer 3 bias and the output DMA are per-group.
"""

from contextlib import ExitStack

import numpy as np

import concourse.bacc as bacc
import concourse.tile as tile
from concourse import mybir
from concourse.bass_utils import run_bass_kernel_spmd

P = 128
N = 512              # edges per tile (one fp32 PSUM bank)
CALL = 512           # indices per dma_gather call (tile-aligned)
IDXW = CALL // 16    # wrapped-index columns per call
TPG = 14             # tiles per group (buffering unit)
NG = 14              # groups
N_TILES = NG * TPG   # 196
NCALLS = N_TILES * N // CALL  # 196 per endpoint
SLOTS = N_TILES * N  # 100352
N_CORES = 8
N_NODES = 50000
BASE = 25000         # gather base row (centered; offsets fit int16)
N_EDGES = 800000
E_CORE = N_EDGES // N_CORES

F16 = mybir.dt.float16
F32 = mybir.dt.float32
I16 = mybir.dt.int16


N_QUEUES = 4  # SWDGE queues; desc-gen parallelizes across Q7 cores


def _build_kernel():
    nc = bacc.Bacc("TRN2", target_bir_lowering=False, debug=False,
                   num_swdge_queues=N_QUEUES,
                   # 4 queues share the SWDGE descriptor carveout; the default
                   # 16KB overflows (clobbered descriptors -> lane-structured
                   # garbage gathers) when desc-gen runs ahead of DMA drain.
                   dynamic_dma_scratch_size=65536)

    tbl = nc.dram_tensor("tbl", [N_NODES, P], F16, kind="ExternalInput")
    colw = nc.dram_tensor("colw", [P, NCALLS * IDXW], I16, kind="ExternalInput")
    roww = nc.dram_tensor("roww", [P, NCALLS * IDXW], I16, kind="ExternalInput")
    w1 = nc.dram_tensor("w1", [P, 1024], F16, kind="ExternalInput")
    w2 = nc.dram_tensor("w2", [P, 512], F16, kind="ExternalInput")
    w3 = nc.dram_tensor("w3", [P, 1], F16, kind="ExternalInput")
    b1 = nc.dram_tensor("b1", [P, 4], F32, kind="ExternalInput")
    b2 = nc.dram_tensor("b2", [P, 1], F32, kind="ExternalInput")
    b3 = nc.dram_tensor("b3", [P, 1], F32, kind="ExternalInput")
    out = nc.dram_tensor("out", [1, N_TILES * N], F32, kind="ExternalOutput")

    Relu = mybir.ActivationFunctionType.Relu
    Identity = mybir.ActivationFunctionType.Identity
    Op = mybir.AluOpType

    with tile.TileContext(nc) as tc, ExitStack() as ctx:
        idxp = ctx.enter_context(tc.tile_pool(name="idxp", bufs=1))
        wp = ctx.enter_context(tc.tile_pool(name="wp", bufs=1))
        gcp = ctx.enter_context(tc.tile_pool(name="gcp", bufs=2 * TPG))
        grp = ctx.enter_context(tc.tile_pool(name="grp", bufs=2 * TPG))
        x1p = ctx.enter_context(tc.tile_pool(name="x1p", bufs=12))
        x2p = ctx.enter_context(tc.tile_pool(name="x2p", bufs=4))
        op = ctx.enter_context(tc.tile_pool(name="op", bufs=2))
        pl1 = ctx.enter_context(tc.tile_pool(name="pl1", bufs=4, space="PSUM"))
        pl2 = ctx.enter_context(tc.tile_pool(name="pl2", bufs=2, space="PSUM"))
        pl3 = ctx.enter_context(tc.tile_pool(name="pl3", bufs=2, space="PSUM"))

        # ---- one-time loads -------------------------------------------
        colw_sb = idxp.tile([P, NCALLS * IDXW], I16)
        roww_sb = idxp.tile([P, NCALLS * IDXW], I16)
        nc.scalar.dma_start(colw_sb[:], colw[:])
        nc.scalar.dma_start(roww_sb[:], roww[:])

        w1_sb = wp.tile([P, 1024], F16)
        w2_sb = wp.tile([P, 512], F16)
        w3_sb = wp.tile([P, 1], F16)
        b1_sb = wp.tile([P, 4], F32)
        b2_sb = wp.tile([P, 1], F32)
        b3_sb = wp.tile([P, 1], F32)
        nc.scalar.dma_start(w1_sb[:], w1[:])
        nc.scalar.dma_start(w2_sb[:], w2[:])
        nc.scalar.dma_start(w3_sb[:], w3[:])
        nc.scalar.dma_start(b1_sb[:], b1[:])
        nc.scalar.dma_start(b2_sb[:], b2[:])
        nc.scalar.dma_start(b3_sb[:], b3[:])

        tblc = tbl[BASE:N_NODES, :]  # centered base; signed offsets

        # Pool-queue guard: SWDGE desc-gen reads index VALUES from SBUF, and
        # the HWDGE idx-load -> Q7 desc-gen dependency has been observed to
        # race on HW. A Pool-engine read of one column of each idx buffer
        # stalls the Pool queue on the idx DMA completion sems, so every
        # later dma_gather is safely behind the loads.
        chk = wp.tile([P, 2], F16)
        nc.gpsimd.tensor_scalar(
            out=chk[:, 0:1], in0=colw_sb[:, 0:1].bitcast(F16),
            scalar1=0.0, scalar2=None, op0=Op.add)
        nc.gpsimd.tensor_scalar(
            out=chk[:, 1:2], in0=roww_sb[:, 0:1].bitcast(F16),
            scalar1=0.0, scalar2=None, op0=Op.add)

        # Sacrificial queue warmup: every observed multi-queue corruption
        # hit only the first ~16 gather calls after start (cold per-queue
        # ring/ucode state). Burn 8 dummy 128-idx gathers per queue into a
        # scratch tile, then serialize, so the transient cannot touch real
        # data.
        warms = [wp.tile([P, 1, 128], F16, name=f"warm{q}")
                 for q in range(N_QUEUES)]
        for w in range(8):
            for q in range(N_QUEUES):
                nc.gpsimd.dma_gather(
                    warms[q][:], tblc, colw_sb[:, 0:8], 128, 128, P,
                    transpose=True, queue_num=q,
                )
        for q in range(N_QUEUES):
            nc.gpsimd.tensor_scalar(
                out=chk[:, 0:1], in0=warms[q][:, 0, 0:1],
                scalar1=0.0, scalar2=None, op0=Op.add)

        qn = [0]

        def emit_group_gathers(g):
            tiles = []
            for j in range(TPG):
                k = g * TPG + j
                gc = gcp.tile([P, 1, N], F16, tag="gc", name=f"gc{k}")
                gr = grp.tile([P, 1, N], F16, tag="gr", name=f"gr{k}")
                for w_sb, gt in ((colw_sb, gc), (roww_sb, gr)):
                    nc.gpsimd.dma_gather(
                        gt[:], tblc,
                        w_sb[:, k * IDXW:(k + 1) * IDXW], CALL, CALL, P,
                        transpose=True, queue_num=qn[0] % N_QUEUES,
                    )
                    qn[0] += 1
                tiles.append((gc, gr))
            return tiles

        gabs = {0: emit_group_gathers(0)}

        for g in range(NG):
            if g + 1 < NG:
                gabs[g + 1] = emit_group_gathers(g + 1)
            tiles = gabs.pop(g)

            o = op.tile([1, TPG * N], F32, tag="o", name=f"o{g}")
            # Consume the two newest tiles first: their 4 gathers are the
            # last call on each SWDGE queue, and per-queue FIFO completion
            # then guarantees every earlier gather of the group has landed
            # (the Tile lane-sem waits assume in-order completion, which 4
            # concurrent queues otherwise break at startup).
            for tt in [TPG - 1, TPG - 2] + list(range(TPG - 2)):
                gc, gr = tiles[tt]
                rc = gc[:, 0, :]
                rr = gr[:, 0, :]

                # layer 1: [E,256] @ [256,512]; K-chunk 0 = col, 1 = row
                x1s = []
                for m in range(4):
                    p1 = pl1.tile([P, N], F32, tag="pl1")
                    nc.tensor.matmul(
                        p1[:], lhsT=w1_sb[:, m * 128:(m + 1) * 128],
                        rhs=rc, start=True, stop=False,
                    )
                    nc.tensor.matmul(
                        p1[:], lhsT=w1_sb[:, 512 + m * 128:512 + (m + 1) * 128],
                        rhs=rr, start=False, stop=True,
                    )
                    x1 = x1p.tile([P, N], F16, tag="x1")
                    if m < 3:
                        nc.scalar.activation(
                            x1[:], p1[:], Relu, bias=b1_sb[:, m:m + 1]
                        )
                    else:
                        nc.vector.tensor_scalar(
                            out=x1[:], in0=p1[:],
                            scalar1=b1_sb[:, m:m + 1], scalar2=0.0,
                            op0=Op.add, op1=Op.max,
                        )
                    x1s.append(x1)

                # layer 2: [E,512] @ [512,128]
                p2 = pl2.tile([P, N], F32, tag="pl2")
                for k in range(4):
                    nc.tensor.matmul(
                        p2[:], lhsT=w2_sb[:, k * 128:(k + 1) * 128],
                        rhs=x1s[k][:], start=(k == 0), stop=(k == 3),
                    )
                x2 = x2p.tile([P, N], F16, tag="x2")
                nc.vector.tensor_scalar(
                    out=x2[:], in0=p2[:],
                    scalar1=b2_sb[:, 0:1], scalar2=0.0,
                    op0=Op.add, op1=Op.max,
                )

                # layer 3: [E,128] @ [128,1]; bias lands row tt of the
                # group staging tile so the output DMA is per-group
                p3 = pl3.tile([1, N], F32, tag="pl3")
                nc.tensor.matmul(p3[:1, :], lhsT=w3_sb[:], rhs=x2[:],
                                 start=True, stop=True)
                nc.scalar.activation(o[0:1, tt * N:(tt + 1) * N], p3[:1, :],
                                     Identity, bias=b3_sb[0:1, 0:1])

            nc.sync.dma_start(
                out[0:1, g * TPG * N:(g + 1) * TPG * N], o[:])

    nc.compile()
    return nc


def _wrap_indices(offs: np.ndarray) -> np.ndarray:
    """[NCALLS*CALL] int16 offsets -> [128, NCALLS*IDXW] wrapped layout.

    dma_gather unwraps each 16-partition group as rearrange("p s -> (s p)")
    per call, so index j of call k sits at [16g + j%16, k*IDXW + j//16],
    replicated over the 8 groups g.
    """
    w = offs.reshape(NCALLS, IDXW, 16).transpose(0, 2, 1)  # [NCALLS, 16, IDXW]
    w = np.tile(w, (1, 8, 1))                              # [NCALLS, 128, IDXW]
    return np.ascontiguousarray(
        w.transpose(1, 0, 2).reshape(P, NCALLS * IDXW))


def _plan_core(col: np.ndarray, row: np.ndarray, base_slot: int):
    """Pad a core's contiguous edge slice to SLOTS and fix call tails.

    The ucode clamps a trailing run of negative int16 indices in each
    dma_gather call, so the last 16 slots of every 896-slot call must hold
    edges with BOTH endpoints >= BASE (offset >= 0). Swap such edges into
    the tail; `so` records each slot's original edge id (-1 = pad).

    Returns (colw [128, NCALLS*IDXW] i16, roww likewise, so [SLOTS] i64).
    """
    n = col.shape[0]
    oc = np.full(SLOTS, 0, np.int64)
    orr = np.full(SLOTS, 0, np.int64)
    so = np.full(SLOTS, -1, np.int64)
    oc[:n] = col - BASE
    orr[:n] = row - BASE
    so[:n] = base_slot + np.arange(n)

    both = (oc >= 0) & (orr >= 0)
    for k in range(NCALLS):
        s = k * CALL
        tail = np.arange(s + CALL - 16, s + CALL)
        tail = tail[~both[tail]]
        if tail.size == 0:
            continue
        cand = s + np.flatnonzero(both[s:s + CALL - 16])
        assert cand.size >= tail.size, "no non-negative tail candidates"
        cand = cand[:tail.size]
        for arr in (oc, orr, so, both):
            arr[tail], arr[cand] = arr[cand], arr[tail]
    assert oc.min() >= -BASE and oc.max() < N_NODES - BASE
    assert orr.min() >= -BASE and orr.max() < N_NODES - BASE
    return (_wrap_indices(oc.astype(np.int16)),
            _wrap_indices(orr.astype(np.int16)), so)


def _prep_shared(emb, W1, b1, W2, b2, W3, b3):
    return {
        "tbl": np.ascontiguousarray(emb.astype(np.float16)),
        "w1": np.ascontiguousarray(
            np.concatenate([W1[:128, :], W1[128:, :]], axis=1)
        ).astype(np.float16),
        "w2": np.ascontiguousarray(
            np.concatenate([W2[k * 128:(k + 1) * 128, :] for k in range(4)],
                           axis=1)
        ).astype(np.float16),
        "w3": W3.astype(np.float16),
        "b1": np.ascontiguousarray(b1.reshape(4, 128).T).astype(np.float32),
        "b2": b2[:, None].astype(np.float32),
        "b3": np.broadcast_to(b3[None, :], (P, 1)).astype(np.float32).copy(),
    }


_NC_CACHE = {}


def _get_nc():
    if "nc" not in _NC_CACHE:
        _NC_CACHE["nc"] = _build_kernel()
    return _NC_CACHE["nc"]


def run(inputs: dict, trace: bool = False):
    """Run the kernel on 8 cores; returns (out [800000,1] f32, results)."""
    emb = np.asarray(inputs["emb"], dtype=np.float32)
    edge_index = np.asarray(inputs["edge_index"])
    shared = _prep_shared(
        emb,
        *[np.asarray(inputs[k], dtype=np.float32)
          for k in ("W1", "b1", "W2", "b2", "W3", "b3")]
    )
    col = np.asarray(edge_index[0], dtype=np.int64)
    row = np.asarray(edge_index[1], dtype=np.int64)

    in_maps = []
    sos = []
    for c in range(N_CORES):
        cw, rw, so = _plan_core(
            col[c * E_CORE:(c + 1) * E_CORE],
            row[c * E_CORE:(c + 1) * E_CORE],
            c * E_CORE,
        )
        in_maps.append(dict(shared, colw=cw, roww=rw))
        sos.append(so)

    nc = _get_nc()
    res = run_bass_kernel_spmd(nc, in_maps, list(range(N_CORES)), trace=trace)
    out = np.empty((N_EDGES,), np.float32)
    for c in range(N_CORES):
        flat = res.results[c]["out"].reshape(-1)
        so = sos[c]
        valid = so >= 0
        out[so[valid]] = flat[valid]
    return out[:, None], res


def kernel(**inputs) -> np.ndarray:
    out, _ = run(inputs, trace=False)
    return out
